# revision 1
# baseline (speedup 1.0000x reference)
"""Trainium2 Bass kernel for a dual-stream cross-attention block.

Reference computation (per batch element b, all fp32 in the oracle):
  Q_l = dwconv3(x_l @ lp1_w1^T + lp1_b1, lp1_w2) + lp1_b2   (and likewise
  Q_r/rp1, V_l/lp2, V_r/rp2)
  attn = Q_l @ Q_r^T * C^-0.5                               (T x T)
  F_r2l = softmax(attn, -1) @ V_r ;  F_l2r = softmax(attn, 1)^T... (bsc)
  out_l = x_l + F_r2l @ lp3_w^T + lp3_b
  out_r = x_r + F_l2r @ rp3_w^T + rp3_b

Sharding: data-parallel over B across the 8 cores (one batch element per
core), params replicated.  Inside a core everything is blocked for the
128x128 PE array.  All large matmuls run in fp8e4m3 with DoubleRow perf
mode (fp32 PSUM accumulation); the ~0.02-scale weights are pre-scaled x16
to stay out of fp8 subnormals, with the 1/16 folded into existing affine
ops.  The k=3 depthwise conv runs on the PE as accumulating diag-matmuls
over zero-padded activations.  exp row-sums come free via the activation
accum_out port.  The V @ w3^T projection is pre-folded (VW) so attention
PV matmuls produce the final projected values directly.  The residual /
epilogue path stays fp32.
"""

import sys

for _p in ("/opt/trn_rl_repo",):
    if _p not in sys.path:
        sys.path.insert(0, _p)

from contextlib import ExitStack

import numpy as np

import concourse.bacc as bacc
import concourse.tile as tile
from concourse import mybir
from concourse.bass_utils import run_bass_kernel_spmd
from concourse.masks import make_identity

B, T, C = 8, 2048, 512
P = 128
NCORES = 8
CCH = C // P      # 4 feature chunks of 128
TCH = T // P      # 16 sequence chunks of 128
NT = 512          # moving-operand tile (free dim)
TT = T // NT      # 4 sequence tiles of 512
SCALE = float(C) ** -0.5

F32 = mybir.dt.float32
BF16 = mybir.dt.bfloat16
FP8 = mybir.dt.float8e4
AX = mybir.AxisListType.X
MULT = mybir.AluOpType.mult
ADD = mybir.AluOpType.add
EXP = mybir.ActivationFunctionType.Exp
IDENT = mybir.ActivationFunctionType.Identity

WNAMES = [
    "lp1_w1", "lp1_b1", "lp1_w2", "lp1_b2",
    "rp1_w1", "rp1_b1", "rp1_w2", "rp1_b2",
    "lp2_w1", "lp2_b1", "lp2_w2", "lp2_b2",
    "rp2_w1", "rp2_b1", "rp2_w2", "rp2_b2",
    "lp3_w", "lp3_b", "rp3_w", "rp3_b",
]


def _build_body(nc, tc, io, ctx):
    """Emit the per-core program.  io maps dram tensor name -> AP."""
    x_l, x_r = io["x_l"], io["x_r"]
    out_l, out_r = io["out_l"], io["out_r"]

    # ---------------- pools (persistent across the kernel) ----------------
    consts = ctx.enter_context(tc.tile_pool(name="consts", bufs=1))
    wp = ctx.enter_context(tc.tile_pool(name="wp", bufs=1))
    qv = ctx.enter_context(tc.tile_pool(name="qv", bufs=1))
    zp = ctx.enter_context(tc.tile_pool(name="zp", bufs=1))
    zstp = ctx.enter_context(tc.tile_pool(name="zstp", bufs=2))
    xload = ctx.enter_context(tc.tile_pool(name="xload", bufs=4))

    ident = consts.tile([P, P], F32)
    make_identity(nc, ident)
    ident_bf = consts.tile([P, P], BF16)
    make_identity(nc, ident_bf)
    ones_row = consts.tile([1, P], F32)
    nc.vector.memset(ones_row, 1.0)
    identrep3 = consts.tile([P, 3, P], BF16)
    nc.gpsimd.memset(identrep3, 0.0)
    nc.gpsimd.affine_select(
        out=identrep3, in_=identrep3, compare_op=mybir.AluOpType.not_equal,
        fill=1.0, base=0, pattern=[[0, 3], [-1, P]], channel_multiplier=1,
    )

    def load_small_params():
        # one strided DMA per tensor: [c] -> [p, chunk], [c, 3] -> [p, chunk, 3]
        small = {}
        for pj in ("lp1", "rp1", "lp2", "rp2"):
            b1t = consts.tile([P, CCH], F32, name=f"{pj}_b1t")
            b2t = consts.tile([P, CCH], F32, name=f"{pj}_b2t")
            w2t = consts.tile([P, CCH, 3], F32, name=f"{pj}_w2t")
            nc.sync.dma_start(b1t, io[f"{pj}_b1"].rearrange("(a b) -> b a", a=CCH))
            nc.sync.dma_start(b2t, io[f"{pj}_b2"].rearrange("(a b) -> b a", a=CCH))
            nc.sync.dma_start(w2t, io[f"{pj}_w2"].rearrange("(a b) c -> b a c", a=CCH))
            small[pj] = (b1t, b2t, w2t)
        return small

    def load_b3bc():
        # broadcast final biases to all partitions, replicated 4x along the
        # free dim so epilogue adds can process 4 sequence chunks in one op
        b3bc = {}
        for nm in ("lp3_b", "rp3_b"):
            b3row = consts.tile([1, C], F32, name=f"{nm}_row")
            nc.sync.dma_start(b3row, io[nm].rearrange("(a b) -> a b", a=1))
            pb = ps_h.tile([P, 2 * NT], F32, tag="h", name=f"{nm}_ps")
            nc.tensor.matmul(pb[:, 0:C], ones_row, b3row, start=True, stop=True)
            bc = consts.tile([P, 4, C], BF16, name=f"{nm}_bc")
            for j in range(4):
                nc.vector.tensor_copy(bc[:, j, :], pb[:, 0:C])
            b3bc[nm] = bc
        return b3bc

    # persistent big tensors
    w3lT = wp.tile([P, CCH, C], FP8)    # lp3_w^T * 16  [c, d]
    w3rT = wp.tile([P, CCH, C], FP8)
    QlT = qv.tile([P, CCH, T], FP8)     # Q^T feature-major [c, t]
    QrT = qv.tile([P, CCH, T], FP8)
    Z1 = zp.tile([P, TCH], F32)
    Z2 = zp.tile([P, TCH], F32)
    rZ1 = zp.tile([P, TCH], F32)
    rZ2 = zp.tile([P, TCH], F32)

    # ---------------- phase 0/1: weights, transposes, projections ----------
    VWr = qv.tile([P, TCH, C], FP8)     # (V_r @ lp3_w^T)*16, natural [s, d]
    VWl = qv.tile([P, TCH, C], FP8)     # (V_l @ rp3_w^T)*16, natural [t, d]

    with ExitStack() as p1:
        ps_h = p1.enter_context(tc.tile_pool(name="ps_h", bufs=2, space="PSUM"))
        wstage = p1.enter_context(tc.tile_pool(name="wstage", bufs=2))
        w1p = p1.enter_context(tc.tile_pool(name="w1p", bufs=1))
        xtp = p1.enter_context(tc.tile_pool(name="xtp", bufs=2))
        hp = p1.enter_context(tc.tile_pool(name="hp", bufs=2))
        vfmp = p1.enter_context(tc.tile_pool(name="vfmp", bufs=1))
        ps_tr = p1.enter_context(tc.tile_pool(name="ps_tr", bufs=2, space="PSUM"))

        def load_wT(dst, w_ap):
            # dst[p, ci, dj*P + j] = 16 * w[dj*P + j, ci*P + p]
            # (x16 keeps the ~0.02-scale weights out of fp8 subnormals; the
            # consumers fold a 1/16 back in).  bf16 cast on gpsimd first so the
            # PE transpose runs at 1 cyc/row.
            for dj in range(CCH):
                wn = wstage.tile([P, C], F32, tag="wstage", name="wn")
                nc.sync.dma_start(wn, w_ap[dj * P : (dj + 1) * P, :])
                wb = wstage.tile([P, C], BF16, tag="wstageb", name="wb")
                nc.gpsimd.tensor_copy(wb, wn)
                pt = ps_tr.tile([P, CCH, P], BF16, tag="ptr", name="ptw")
                for ci in range(CCH):
                    nc.tensor.transpose(
                        pt[:, ci, :], wb[:, ci * P : (ci + 1) * P], ident_bf
                    )
                nc.vector.tensor_scalar_mul(
                    dst[:, :, dj * P : (dj + 1) * P], pt, 16.0
                )

        w1T = {}
        for pj in ("lp1", "rp1", "lp2", "rp2"):
            w1T[pj] = w1p.tile([P, CCH, C], FP8, name=f"{pj}_w1T")
            load_wT(w1T[pj], io[f"{pj}_w1"])

        # per-channel conv taps as diagonal matrices:
        # D[p, dc, k, y] = (p == y) * w2[dc*P + p, k]
        dtaps = {}
        for pj in ("lp1", "rp1", "lp2", "rp2"):
            D = w1p.tile([P, CCH, 3, P], BF16, name=f"{pj}_D")
            for dc in range(CCH):
                wrow = wstage.tile([1, 3, P], F32, tag="wrow", name="wrow")
                nc.sync.dma_start(
                    wrow,
                    io[f"{pj}_w2"][dc * P : (dc + 1) * P, :]
                    .rearrange("(a y) c -> a c y", a=1),
                )
                pw = ps_tr.tile([P, 3 * P], F32, tag="pw", name="pw")
                nc.tensor.matmul(pw, ones_row, wrow.rearrange("a b c -> a (b c)"),
                                 start=True, stop=True)
                nc.vector.tensor_mul(
                    D[:, dc, :, :].rearrange("a b c -> a (b c)"), identrep3
                    .rearrange("a b c -> a (b c)"), pw,
                )
            dtaps[pj] = D

        def load_xT(dst, x_ap):
            # dst[p, ci, tc*P + j] = x[tc*P + j, ci*P + p]
            for tcn in range(TCH):
                xn = xload.tile([P, C], F32, tag="xl", name="xn")
                nc.sync.dma_start(xn, x_ap[tcn * P : (tcn + 1) * P, :])
                xb = wstage.tile([P, C], BF16, tag="wstageb", name="xb")
                nc.gpsimd.tensor_copy(xb, xn)
                pt = ps_tr.tile([P, CCH, P], BF16, tag="ptr", name="ptx")
                for ci in range(CCH):
                    nc.tensor.transpose(
                        pt[:, ci, :], xb[:, ci * P : (ci + 1) * P], ident_bf
                    )
                nc.scalar.copy(dst[:, :, tcn * P : (tcn + 1) * P], pt)

        def project(dst, xT, pj):
            """dst[:, dc, t] = depthwise-conv3(x @ w1^T + b1)^T in [d, t], fp8.

            Pointwise matmuls use x16-scaled fp8 weights (1/16 folded into the
            psum->sbuf bias copy).  The k=3 depthwise conv runs on the PE as 3
            accumulating diag-matmuls over a zero-padded H, so no elementwise
            shift ops are needed."""
            b1t, b2t, w2t = small[pj]
            D = dtaps[pj]
            W2c = 2 * NT
            H = hp.tile([P, CCH, T + 2], BF16, tag="H", name=f"H_{pj}")
            nc.vector.memset(H[:, :, 0:1], 0.0)
            nc.vector.memset(H[:, :, T + 1 : T + 2], 0.0)
            for dc in range(CCH):
                for tth in range(2):
                    ph = ps_h.tile([P, W2c], F32, tag="h", name="ph")
                    for half in range(2):
                        tt = 2 * tth + half
                        tsl = slice(tt * NT, (tt + 1) * NT)
                        for cc2 in range(CCH // 2):
                            nc.tensor.matmul(
                                ph[:, half * NT : (half + 1) * NT],
                                w1T[pj][:, 2 * cc2 : 2 * cc2 + 2,
                                        dc * P : (dc + 1) * P],
                                xT[:, 2 * cc2 : 2 * cc2 + 2, tsl],
                                start=(cc2 == 0),
                                stop=(cc2 == CCH // 2 - 1),
                                perf_mode=mybir.MatmulPerfMode.DoubleRow,
                            )
                    nc.vector.tensor_scalar(
                        H[:, dc, 1 + tth * W2c : 1 + (tth + 1) * W2c], ph,
                        1.0 / 16.0, b1t[:, dc : dc + 1], op0=MULT, op1=ADD,
                    )
                for tth in range(2):
                    pq = ps_h.tile([P, W2c], F32, tag="h", name="pq")
                    for half in range(2):
                        tt = 2 * tth + half
                        for k in range(3):
                            nc.tensor.matmul(
                                pq[:, half * NT : (half + 1) * NT],
                                D[:, dc, k, :],
                                H[:, dc, tt * NT + k : tt * NT + k + NT],
                                start=(k == 0),
                                stop=(k == 2),
                            )
                    nc.scalar.activation(
                        dst[:, dc, tth * W2c : (tth + 1) * W2c], pq, IDENT,
                        bias=b2t[:, dc : dc + 1], scale=1.0,
                    )

        def vw_precompute(dst, vfm, w3T):
            # dst[p, sc, d] = 16 * sum_c V[sc*P + p, c] w3[d, c]
            for sc2 in range(TCH // 2):
                pv = ps_h.tile([P, 2 * NT], F32, tag="h", name="pvw")
                for half in range(2):
                    sc = 2 * sc2 + half
                    for cc2 in range(CCH // 2):
                        nc.tensor.matmul(
                            pv[:, half * C : (half + 1) * C],
                            vfm[:, 2 * cc2 : 2 * cc2 + 2, sc * P : (sc + 1) * P],
                            w3T[:, 2 * cc2 : 2 * cc2 + 2, :],
                            start=(cc2 == 0),
                            stop=(cc2 == CCH // 2 - 1),
                            perf_mode=mybir.MatmulPerfMode.DoubleRow,
                        )
                nc.scalar.copy(dst[:, 2 * sc2 : 2 * sc2 + 2, :], pv)

        # Q projections first so the attention score pass unblocks as early
        # as possible; V projections + their VW matmuls fill in behind it.
        xlT = xtp.tile([P, CCH, T], FP8, tag="xT", name="xlT")
        load_xT(xlT, x_l)
        xrT = xtp.tile([P, CCH, T], FP8, tag="xT", name="xrT")
        load_xT(xrT, x_r)
        small = load_small_params()
        b3bc = load_b3bc()
        load_wT(w3lT, io["lp3_w"])
        load_wT(w3rT, io["rp3_w"])
        project(QlT, xlT, "lp1")
        project(QrT, xrT, "rp1")
        VlT = vfmp.tile([P, CCH, T], FP8, tag="vfm", name="VlT")
        project(VlT, xlT, "lp2")
        vw_precompute(VWl, VlT, w3rT)
        VrT = vfmp.tile([P, CCH, T], FP8, tag="vfm", name="VrT")
        project(VrT, xrT, "rp2")
        vw_precompute(VWr, VrT, w3lT)

    # ---------------- phases 2/3: attention ----------------
    ps_s = ctx.enter_context(tc.tile_pool(name="ps_s", bufs=2, space="PSUM"))
    ps_pu = ctx.enter_context(tc.tile_pool(name="ps_pu", bufs=2, space="PSUM"))
    ep1 = ctx.enter_context(tc.tile_pool(name="ep1", bufs=1))
    ep2 = ctx.enter_context(tc.tile_pool(name="ep2", bufs=1))
    u2p = ctx.enter_context(tc.tile_pool(name="u2p", bufs=1))
    outp = ctx.enter_context(tc.tile_pool(name="outp", bufs=2))

    def s_pass(E, Z, qrow, qcol):
        """E[:, rc, s] = exp(scale * qrow^T qcol), Z[:, rc] = row sums.

        Emitted column-major (st outer) so each score column is complete
        early and the downstream PV accumulation can start behind it."""
        W2 = 2 * NT
        zst = zstp.tile([P, TCH, T // W2], F32, tag="zst", name="zst")
        for st in range(T // W2):
            for rc in range(TCH):
                ps = ps_s.tile([P, W2], F32, tag="s2w", name="ps_s")
                for half in range(2):
                    hsl = slice(st * W2 + half * NT, st * W2 + (half + 1) * NT)
                    for cc2 in range(CCH // 2):
                        nc.tensor.matmul(
                            ps[:, half * NT : (half + 1) * NT],
                            qrow[:, 2 * cc2 : 2 * cc2 + 2, rc * P : (rc + 1) * P],
                            qcol[:, 2 * cc2 : 2 * cc2 + 2, hsl],
                            start=(cc2 == 0),
                            stop=(cc2 == CCH // 2 - 1),
                            perf_mode=mybir.MatmulPerfMode.DoubleRow,
                        )
                nc.scalar.activation(
                    E[:, rc, st * W2 : (st + 1) * W2], ps, EXP, scale=SCALE,
                    accum_out=zst[:, rc, st : st + 1],
                )
        nc.vector.reduce_sum(Z, zst, axis=AX)

    def pv(E, VW, sink):
        """sink(tc2, psum[P, 2, C]) with psum[t', j, d] =
        sum_s E[s, (2 tc2 + j) P + t'] VW[s, d]."""
        for tc2 in range(TCH // 2):
            pu = ps_pu.tile([P, 2, C], F32, tag="pu", name="pu")
            for j in range(2):
                tcn = 2 * tc2 + j
                for kc2 in range(TCH // 2):
                    nc.tensor.matmul(
                        pu[:, j, :],
                        E[:, 2 * kc2 : 2 * kc2 + 2, tcn * P : (tcn + 1) * P],
                        VW[:, 2 * kc2 : 2 * kc2 + 2, :],
                        start=(kc2 == 0),
                        stop=(kc2 == TCH // 2 - 1),
                        perf_mode=mybir.MatmulPerfMode.DoubleRow,
                    )
            sink(tc2, pu)

    # E1 in [t, s] layout (+ Z1), consumed by the l->r direction
    E1 = ep1.tile([P, TCH, T], FP8, name="E1")
    s_pass(E1, Z1, QlT, QrT)
    nc.vector.reciprocal(rZ1, Z1)
    nc.vector.tensor_scalar_mul(rZ1, rZ1, 1.0 / 16.0)

    # l->r direction: out_r[s, d] = x_r + (sum_t E1[t,s] VW_l[t,d]) / (16 Z2)
    # Z2 isn't known yet, so stash the unnormalized projected values.
    U2st = u2p.tile([P, TCH, C], BF16)

    def sink_stash(idx, pu):
        nc.scalar.copy(U2st[:, 2 * idx : 2 * idx + 2, :], pu)

    pv(E1, VWl, sink_stash)

    # E2 in [s, t] layout (+ Z2), independent slot so phases overlap freely
    E2 = ep2.tile([P, TCH, T], FP8, name="E2")
    s_pass(E2, Z2, QrT, QlT)
    nc.vector.reciprocal(rZ2, Z2)
    nc.vector.tensor_scalar_mul(rZ2, rZ2, 1.0 / 16.0)

    # out_r epilogue from the stash: 4 sequence chunks per load/add/store
    for g in range(TCH // 4):
        gsl = slice(g * 4 * P, (g + 1) * 4 * P)
        xr = xload.tile([P, 4, C], F32, tag="xl4", name="xr_ep")
        nc.sync.dma_start(xr, x_r[gsl, :].rearrange("(a p) c -> p a c", p=P))
        o = outp.tile([P, 4, C], F32, tag="o", name="o_r")
        nc.gpsimd.tensor_add(o, xr, b3bc["rp3_b"])
        for j in range(4):
            sc = 4 * g + j
            nc.vector.scalar_tensor_tensor(
                o[:, j, :], U2st[:, sc, :], rZ2[:, sc : sc + 1], o[:, j, :],
                op0=MULT, op1=ADD,
            )
        nc.sync.dma_start(out_r[gsl, :].rearrange("(a p) c -> p a c", p=P), o)

    # r->l direction: direct epilogue; 4 chunks (two pv pairs) per group
    stage = {}

    def sink_l(idx, pu):
        g, phase = divmod(idx, 2)
        if phase == 0:
            gsl = slice(g * 4 * P, (g + 1) * 4 * P)
            xl = xload.tile([P, 4, C], F32, tag="xl4", name="xl_ep")
            nc.sync.dma_start(xl, x_l[gsl, :].rearrange("(a p) c -> p a c", p=P))
            o = outp.tile([P, 4, C], F32, tag="o", name="o_l")
            nc.gpsimd.tensor_add(o, xl, b3bc["lp3_b"])
            stage[g] = o
        o = stage[g]
        for j in range(2):
            sc = 2 * idx + j
            nc.vector.scalar_tensor_tensor(
                o[:, 2 * phase + j, :], pu[:, j, :], rZ1[:, sc : sc + 1],
                o[:, 2 * phase + j, :], op0=MULT, op1=ADD,
            )
        if phase == 1:
            gsl = slice(g * 4 * P, (g + 1) * 4 * P)
            nc.sync.dma_start(
                out_l[gsl, :].rearrange("(a p) c -> p a c", p=P), o
            )

    pv(E2, VWr, sink_l)


def build_nc():
    nc = bacc.Bacc(
        "TRN2",
        target_bir_lowering=False,
        debug=False,
        enable_asserts=False,
        num_devices=NCORES,
    )
    io = {}
    io["x_l"] = nc.dram_tensor("x_l", [T, C], F32, kind="ExternalInput").ap()
    io["x_r"] = nc.dram_tensor("x_r", [T, C], F32, kind="ExternalInput").ap()
    for nm in WNAMES:
        if nm.endswith("_w1") or nm in ("lp3_w", "rp3_w"):
            shape = [C, C]
        elif nm.endswith("_w2"):
            shape = [C, 3]
        else:
            shape = [C]
        io[nm] = nc.dram_tensor(nm, shape, F32, kind="ExternalInput").ap()
    io["out_l"] = nc.dram_tensor("out_l", [T, C], F32, kind="ExternalOutput").ap()
    io["out_r"] = nc.dram_tensor("out_r", [T, C], F32, kind="ExternalOutput").ap()

    with tile.TileContext(nc) as tc:
        with ExitStack() as ctx:
            _build_body(nc, tc, io, ctx)
    nc.compile()
    return nc


_NC_CACHE = None


def _get_nc():
    global _NC_CACHE
    if _NC_CACHE is None:
        _NC_CACHE = build_nc()
    return _NC_CACHE


def make_in_maps(inputs):
    ins = {k: np.ascontiguousarray(np.asarray(v, dtype=np.float32)) for k, v in inputs.items()}
    in_maps = []
    for c in range(NCORES):
        m = {"x_l": ins["x_l"][c], "x_r": ins["x_r"][c]}
        for nm in WNAMES:
            m[nm] = ins[nm]
        in_maps.append(m)
    return in_maps


def run(inputs, **kw):
    nc = _get_nc()
    res = run_bass_kernel_spmd(nc, make_in_maps(inputs), list(range(NCORES)), **kw)
    out_l = np.stack([res.results[c]["out_l"] for c in range(NCORES)])
    out_r = np.stack([res.results[c]["out_r"] for c in range(NCORES)])
    return (out_l, out_r), res


def kernel(**inputs):
    outs, _ = run(inputs)
    return outs

